# revision 10
# baseline (speedup 1.0000x reference)
"""BatchTopK Tied SAE kernel for 8 Trainium2 NeuronCores.

Strategy (tensor-parallel over d_hidden):
  - Each core owns a 2048-column shard of W (d_hidden 16384 / 8).
  - Launch 1 (encode): f_shard = relu(x @ W_shard + b_enc_shard), computed
    h-major (fT layout [2048, 4096]) so the decode matmul can consume it
    without any transposes. Stationary operand = W chunks (natural layout),
    moving operand = xT chunks (host-pretransposed x). Single-pass float32r
    (FP22) matmuls. Also emits a strided value sample (every 8th element)
    used by the host to pick a conservative global top-k threshold.
  - Host: picks cut = value at sample-rank (n_keep + margin) so that
    #elements >= cut is guaranteed (whp, verified) to exceed n_keep.
  - Launch 2 (mask + decode): f_masked = f * (f >= cut); writes f_masked
    (h-major) and recon_partial = f_masked @ W_shard.T (fp22 matmuls,
    contraction over the h shard). Host sums the 8 recon partials.
  - Host exactness fixup: the top-n_keep selection boundary is resolved
    exactly on the host: t = n-th largest masked value; elements within a
    small window around t are recomputed in float64 (a few hundred dot
    products) to reproduce the reference's exact fp32 top-k selection; all
    sub-threshold survivors of the conservative cut are zeroed out of
    f_topk and their rank-1 contributions are subtracted from recon.

Self-contained: hardcodes shapes B=4096, D_IN=2048, D_HIDDEN=16384, 8 cores.
"""

import os

import numpy as np

import concourse.bass as bass
import concourse.bacc as bacc
import concourse.mybir as mybir
from concourse.bass_utils import run_bass_kernel_spmd
from concourse.tile import TileContext

F32 = mybir.dt.float32
F32R = mybir.dt.float32r

B = 4096
D_IN = 2048
H = 16384
NCORES = 8
HS = H // NCORES  # 2048 hidden columns per core

# encode tiling
KC = D_IN // 128  # 16 contraction chunks
BT = 512  # batch tile (moving free dim)
NBT = B // BT  # 8
NHG = 2  # h-shard halves (W residency)
HTH = HS // NHG // 128  # 8 h-tiles of 128 per half
# decode tiling
NDH = 2  # d_in halves (W.T residency)
DW = D_IN // NDH  # 1024
KH = HS // 128  # 16 contraction chunks over h shard

SAMPLE_STRIDE = 8
NSAMP_COL = B // SAMPLE_STRIDE  # 512

_CACHE = {}

# filled by kernel(): [encode BassKernelResults, decode BassKernelResults]
LAST_RESULTS = []


def _build_encode():
    nc = bacc.Bacc("TRN2")
    xT = nc.dram_tensor("xT", [D_IN, B], F32R, kind="ExternalInput")
    w = nc.dram_tensor("w", [D_IN, HS], F32R, kind="ExternalInput")
    benc = nc.dram_tensor("benc", [128, HS // 128], F32, kind="ExternalInput")
    f22 = nc.dram_tensor("f22", [HS, B], F32, kind="ExternalOutput")
    samp = nc.dram_tensor("samp", [HS, NSAMP_COL], F32, kind="ExternalOutput")

    with TileContext(nc) as tc:
        with (
            tc.tile_pool(name="wp", bufs=1) as wp,
            tc.tile_pool(name="xp", bufs=2) as xp,
            tc.tile_pool(name="fp", bufs=4) as fp,
            tc.tile_pool(name="sp", bufs=4) as sp,
            tc.tile_pool(name="bp", bufs=1) as bp,
            tc.tile_pool(name="pp", bufs=6, space="PSUM") as pp,
        ):
            b_sb = bp.tile([128, HS // 128], F32)
            nc.sync.dma_start(b_sb, benc[:, :])
            for hg in range(NHG):
                w_sb = wp.tile([128, KC * (HS // NHG)], F32R, tag="w")
                hw = HS // NHG  # 1024
                for k in range(KC):
                    nc.sync.dma_start(
                        w_sb[:, k * hw : (k + 1) * hw],
                        w[k * 128 : (k + 1) * 128, hg * hw : (hg + 1) * hw],
                    )
                for bt in range(NBT):
                    x_sb = xp.tile([128, KC * BT], F32R, tag="x")
                    for k in range(KC):
                        nc.sync.dma_start(
                            x_sb[:, k * BT : (k + 1) * BT],
                            xT[k * 128 : (k + 1) * 128, bt * BT : (bt + 1) * BT],
                        )
                    for ht in range(HTH):
                        ps = pp.tile([128, BT], F32, tag="ps")
                        for k in range(KC):
                            nc.tensor.matmul(
                                ps,
                                lhsT=w_sb[
                                    :, k * hw + ht * 128 : k * hw + (ht + 1) * 128
                                ],
                                rhs=x_sb[:, k * BT : (k + 1) * BT],
                                start=(k == 0),
                                stop=(k == KC - 1),
                            )
                        f_sb = fp.tile([128, BT], F32, tag="f")
                        hcol = hg * HTH + ht
                        nc.scalar.activation(
                            f_sb,
                            ps,
                            mybir.ActivationFunctionType.Relu,
                            bias=b_sb[:, hcol : hcol + 1],
                            scale=1.0,
                        )
                        s_sb = sp.tile([128, BT // SAMPLE_STRIDE], F32, tag="s")
                        nc.vector.tensor_copy(s_sb, f_sb[:, ::SAMPLE_STRIDE])
                        h0 = hg * hw + ht * 128
                        nc.sync.dma_start(
                            f22[h0 : h0 + 128, bt * BT : (bt + 1) * BT], f_sb
                        )
                        nbs = BT // SAMPLE_STRIDE
                        nc.sync.dma_start(
                            samp[h0 : h0 + 128, bt * nbs : (bt + 1) * nbs], s_sb
                        )
    if not nc.is_finalized():
        nc.finalize()
    return nc


def _build_decode():
    nc = bacc.Bacc("TRN2")
    f22 = nc.dram_tensor("f22", [HS, B], F32, kind="ExternalInput")
    wt = nc.dram_tensor("wt", [HS, D_IN], F32R, kind="ExternalInput")
    cutv = nc.dram_tensor("cutv", [128, 1], F32, kind="ExternalInput")
    ftopk = nc.dram_tensor("ftopk", [HS, B], F32R, kind="ExternalOutput")
    recon = nc.dram_tensor("recon", [B, D_IN], F32, kind="ExternalOutput")

    with TileContext(nc) as tc:
        with (
            tc.tile_pool(name="wp", bufs=1) as wp,
            tc.tile_pool(name="mp", bufs=1) as mp,
            tc.tile_pool(name="gp", bufs=2) as gp,
            tc.tile_pool(name="cp", bufs=1) as cp,
            tc.tile_pool(name="op", bufs=4) as op,
            tc.tile_pool(name="pp", bufs=4, space="PSUM") as pp,
        ):
            cut_sb = cp.tile([128, 1], F32)
            nc.sync.dma_start(cut_sb, cutv[:, :])
            for dh in range(NDH):
                wt_sb = wp.tile([128, KH * DW], F32R, tag="wt")
                for k in range(KH):
                    nc.sync.dma_start(
                        wt_sb[:, k * DW : (k + 1) * DW],
                        wt[k * 128 : (k + 1) * 128, dh * DW : (dh + 1) * DW],
                    )
                for bt in range(NBT):
                    m_sb = mp.tile([128, KH * BT], F32, tag="m")
                    for k in range(KH):
                        nc.sync.dma_start(
                            m_sb[:, k * BT : (k + 1) * BT],
                            f22[k * 128 : (k + 1) * 128, bt * BT : (bt + 1) * BT],
                        )
                    # masked = (f >= cut) * f, written to the f32r staging
                    # tile that feeds the PE (and the ftopk output)
                    g_sb = gp.tile([128, KH * BT], F32R, tag="g")
                    nc.vector.scalar_tensor_tensor(
                        out=g_sb,
                        in0=m_sb,
                        scalar=cut_sb[:, 0:1],
                        in1=m_sb,
                        op0=mybir.AluOpType.is_ge,
                        op1=mybir.AluOpType.mult,
                    )
                    if dh == 0:
                        for k in range(KH):
                            nc.sync.dma_start(
                                ftopk[
                                    k * 128 : (k + 1) * 128, bt * BT : (bt + 1) * BT
                                ],
                                g_sb[:, k * BT : (k + 1) * BT],
                            )
                    for sb in range(BT // 128):
                        pts = [
                            pp.tile([128, 512], F32, tag=f"ps{dt}", name=f"pts{dt}")
                            for dt in range(DW // 512)
                        ]
                        for k in range(KH):
                            lhsT = g_sb[
                                :, k * BT + sb * 128 : k * BT + (sb + 1) * 128
                            ]
                            for dt in range(DW // 512):
                                nc.tensor.matmul(
                                    pts[dt],
                                    lhsT=lhsT,
                                    rhs=wt_sb[
                                        :, k * DW + dt * 512 : k * DW + (dt + 1) * 512
                                    ],
                                    start=(k == 0),
                                    stop=(k == KH - 1),
                                )
                        for dt in range(DW // 512):
                            o_sb = op.tile([128, 512], F32, tag="o")
                            nc.scalar.copy(o_sb, pts[dt])
                            r0 = bt * BT + sb * 128
                            c0 = dh * DW + dt * 512
                            nc.sync.dma_start(
                                recon[r0 : r0 + 128, c0 : c0 + 512], o_sb
                            )
    if not nc.is_finalized():
        nc.finalize()
    return nc


def _get(name):
    if name not in _CACHE:
        _CACHE[name] = _build_encode() if name == "enc" else _build_decode()
    return _CACHE[name]


def _run_spmd(nc, in_maps):
    trace = bool(os.environ.get("BASS_TRACE"))
    res = run_bass_kernel_spmd(nc, in_maps, core_ids=list(range(NCORES)), trace=trace)
    LAST_RESULTS.append(res)
    return res.results


def kernel(x, W, b_enc, b_dec, k):
    LAST_RESULTS.clear()
    x = np.asarray(x, dtype=np.float32)
    W = np.asarray(W, dtype=np.float32)
    b_enc = np.asarray(b_enc, dtype=np.float32)
    b_dec = np.asarray(b_dec, dtype=np.float32)
    n_keep = int(k) * B

    xT = np.ascontiguousarray(x.T)
    w_shards = [np.ascontiguousarray(W[:, c * HS : (c + 1) * HS]) for c in range(NCORES)]
    wt_shards = [np.ascontiguousarray(ws.T) for ws in w_shards]
    benc_shards = [
        np.ascontiguousarray(
            b_enc[c * HS : (c + 1) * HS].reshape(HS // 128, 128).T
        )
        for c in range(NCORES)
    ]

    # ---- launch 1: encode ----
    enc_in = [
        {"xT": xT, "w": w_shards[c], "benc": benc_shards[c]} for c in range(NCORES)
    ]
    enc_out = _run_spmd(_get("enc"), enc_in)
    f22 = [enc_out[c]["f22"] for c in range(NCORES)]
    samples = np.concatenate([enc_out[c]["samp"].ravel() for c in range(NCORES)])

    # ---- host: conservative threshold from samples ----
    margin = max(8000, n_keep // 32)
    srank = min((n_keep + margin) // SAMPLE_STRIDE, samples.size - 1)
    cut = float(np.partition(samples, samples.size - 1 - srank)[samples.size - 1 - srank])
    cut -= 2e-5
    cut = max(cut, 1e-30)

    for _attempt in range(4):
        out = _decode_and_fix(x, W, b_dec, n_keep, f22, wt_shards, cut)
        if out is not None:
            return out
        cut *= 0.25  # too few survivors; retry with a much lower threshold
    raise RuntimeError("batch top-k threshold selection failed")


def _decode_and_fix(x, W, b_dec, n_keep, f22, wt_shards, cut):
    cut_arr = np.full((128, 1), cut, dtype=np.float32)
    dec_in = [
        {"f22": f22[c], "wt": wt_shards[c], "cutv": cut_arr} for c in range(NCORES)
    ]
    dec_out = _run_spmd(_get("dec"), dec_in)
    # [HS, B] h-major shards (copy: PJRT buffers are read-only)
    ft = [np.array(dec_out[c]["ftopk"]) for c in range(NCORES)]
    recon = dec_out[0]["recon"].astype(np.float32, copy=True)
    for c in range(1, NCORES):
        recon += dec_out[c]["recon"]

    # ---- host: exact top-n_keep boundary fixup ----
    nz_h, nz_b, nz_v, nz_c = [], [], [], []
    for c in range(NCORES):
        hh, bb = np.nonzero(ft[c])
        nz_h.append(hh)
        nz_b.append(bb)
        nz_v.append(ft[c][hh, bb])
        nz_c.append(np.full(hh.shape, c, dtype=np.int32))
    v = np.concatenate(nz_v)
    m = v.size
    if m < n_keep:
        return None
    hl = np.concatenate(nz_h)
    bl = np.concatenate(nz_b)
    cl = np.concatenate(nz_c)
    hg = cl.astype(np.int64) * HS + hl  # global hidden index

    t = float(np.partition(v, m - n_keep)[m - n_keep])  # n-th largest f22 value
    WIN = 1.2e-4  # covers fp22 matmul noise + possible fp22 storage rounding
    if not (cut <= t - WIN):
        return None
    definite = v > t + WIN
    n_def = int(definite.sum())
    bmask = ~definite & (v >= t - WIN)
    n_bnd = int(bmask.sum())
    if n_def > n_keep or n_def + n_bnd < n_keep:
        return None

    # recompute boundary candidates exactly; order like the reference fp32 topk
    bi = np.nonzero(bmask)[0]
    x64 = x.astype(np.float64)
    W64 = W.astype(np.float64)
    exact = np.empty(bi.size, dtype=np.float64)
    for j, i in enumerate(bi):
        exact[j] = np.dot(x64[bl[i]], W64[:, hg[i]])
    exact32 = np.maximum(exact, 0.0).astype(np.float32)
    flat_idx = bl[bi].astype(np.int64) * H + hg[bi]
    order = np.lexsort((flat_idx, -exact32.astype(np.float64)))
    keep_bnd = bi[order[: n_keep - n_def]]

    keep = np.zeros(m, dtype=bool)
    keep[definite] = True
    keep[keep_bnd] = True
    drop = np.nonzero(~keep)[0]

    # zero dropped entries in the f_topk shards
    for c in range(NCORES):
        sel = drop[cl[drop] == c]
        if sel.size:
            ft[c][hl[sel], bl[sel]] = 0.0

    # subtract dropped rank-1 contributions from recon
    if drop.size:
        contrib = v[drop][:, None].astype(np.float32) * W[:, hg[drop]].T
        np.add.at(recon, bl[drop], -contrib)

    recon += b_dec[None, :]

    f_topk = np.empty((B, H), dtype=np.float32)
    for c in range(NCORES):
        f_topk[:, c * HS : (c + 1) * HS] = ft[c].T
    return recon, f_topk


# revision 11
# speedup vs baseline: 1.1067x; 1.1067x over previous
"""BatchTopK Tied SAE kernel for 8 Trainium2 NeuronCores.

Strategy (tensor-parallel over d_hidden):
  - Each core owns a 2048-column shard of W (d_hidden 16384 / 8).
  - Launch 1 (encode): f_shard = relu(x @ W_shard + b_enc_shard), computed
    h-major (fT layout [2048, 4096]) so the decode matmul can consume it
    without any transposes. Stationary operand = W chunks (natural layout),
    moving operand = xT chunks (host-pretransposed x). Single-pass float32r
    (FP22) matmuls. Also emits a strided value sample (every 8th element)
    used by the host to pick a conservative global top-k threshold.
  - Host: picks cut = value at sample-rank (n_keep + margin) so that
    #elements >= cut is guaranteed (whp, verified) to exceed n_keep.
  - Launch 2 (mask + decode): f_masked = f * (f >= cut); writes f_masked
    (h-major) and recon_partial = f_masked @ W_shard.T (fp22 matmuls,
    contraction over the h shard). Host sums the 8 recon partials.
  - Host exactness fixup: the top-n_keep selection boundary is resolved
    exactly on the host: t = n-th largest masked value; elements within a
    small window around t are recomputed in float64 (a few hundred dot
    products) to reproduce the reference's exact fp32 top-k selection; all
    sub-threshold survivors of the conservative cut are zeroed out of
    f_topk and their rank-1 contributions are subtracted from recon.

Self-contained: hardcodes shapes B=4096, D_IN=2048, D_HIDDEN=16384, 8 cores.
"""

import os

import numpy as np

import concourse.bass as bass
import concourse.bacc as bacc
import concourse.mybir as mybir
from concourse.bass_utils import run_bass_kernel_spmd
from concourse.tile import TileContext

F32 = mybir.dt.float32
F32R = mybir.dt.float32r

B = 4096
D_IN = 2048
H = 16384
NCORES = 8
HS = H // NCORES  # 2048 hidden columns per core

# encode tiling
KC = D_IN // 128  # 16 contraction chunks
BT = 512  # batch tile (moving free dim)
NBT = B // BT  # 8
NHG = 2  # h-shard halves (W residency)
HTH = HS // NHG // 128  # 8 h-tiles of 128 per half
# decode tiling
NDH = 2  # d_in halves (W.T residency)
DW = D_IN // NDH  # 1024
KH = HS // 128  # 16 contraction chunks over h shard

SAMPLE_STRIDE = 8
NSAMP_COL = B // SAMPLE_STRIDE  # 512

_CACHE = {}

# filled by kernel(): [encode BassKernelResults, decode BassKernelResults]
LAST_RESULTS = []


def _build_encode():
    nc = bacc.Bacc("TRN2")
    xT = nc.dram_tensor("xT", [D_IN, B], F32R, kind="ExternalInput")
    w = nc.dram_tensor("w", [D_IN, HS], F32R, kind="ExternalInput")
    benc = nc.dram_tensor("benc", [128, HS // 128], F32, kind="ExternalInput")
    f22 = nc.dram_tensor("f22", [HS, B], F32, kind="ExternalOutput")
    samp = nc.dram_tensor("samp", [HS, NSAMP_COL], F32, kind="ExternalOutput")

    with TileContext(nc) as tc:
        with (
            tc.tile_pool(name="wp", bufs=1) as wp,
            tc.tile_pool(name="xp", bufs=2) as xp,
            tc.tile_pool(name="fp", bufs=4) as fp,
            tc.tile_pool(name="sp", bufs=4) as sp,
            tc.tile_pool(name="bp", bufs=1) as bp,
            tc.tile_pool(name="pp", bufs=6, space="PSUM") as pp,
        ):
            b_sb = bp.tile([128, HS // 128], F32)
            nc.sync.dma_start(b_sb, benc[:, :])
            for hg in range(NHG):
                w_sb = wp.tile([128, KC * (HS // NHG)], F32R, tag="w")
                hw = HS // NHG  # 1024
                for k in range(KC):
                    nc.sync.dma_start(
                        w_sb[:, k * hw : (k + 1) * hw],
                        w[k * 128 : (k + 1) * 128, hg * hw : (hg + 1) * hw],
                    )
                for bt in range(NBT):
                    x_sb = xp.tile([128, KC * BT], F32R, tag="x")
                    for k in range(KC):
                        nc.sync.dma_start(
                            x_sb[:, k * BT : (k + 1) * BT],
                            xT[k * 128 : (k + 1) * 128, bt * BT : (bt + 1) * BT],
                        )
                    for ht in range(HTH):
                        ps = pp.tile([128, BT], F32, tag="ps")
                        for k in range(KC):
                            nc.tensor.matmul(
                                ps,
                                lhsT=w_sb[
                                    :, k * hw + ht * 128 : k * hw + (ht + 1) * 128
                                ],
                                rhs=x_sb[:, k * BT : (k + 1) * BT],
                                start=(k == 0),
                                stop=(k == KC - 1),
                            )
                        f_sb = fp.tile([128, BT], F32, tag="f")
                        hcol = hg * HTH + ht
                        nc.scalar.activation(
                            f_sb,
                            ps,
                            mybir.ActivationFunctionType.Relu,
                            bias=b_sb[:, hcol : hcol + 1],
                            scale=1.0,
                        )
                        s_sb = sp.tile([128, BT // SAMPLE_STRIDE], F32, tag="s")
                        nc.vector.tensor_copy(s_sb, f_sb[:, ::SAMPLE_STRIDE])
                        h0 = hg * hw + ht * 128
                        nc.sync.dma_start(
                            f22[h0 : h0 + 128, bt * BT : (bt + 1) * BT], f_sb
                        )
                        nbs = BT // SAMPLE_STRIDE
                        nc.sync.dma_start(
                            samp[h0 : h0 + 128, bt * nbs : (bt + 1) * nbs], s_sb
                        )
    if not nc.is_finalized():
        nc.finalize()
    return nc


def _build_decode():
    nc = bacc.Bacc("TRN2")
    f22 = nc.dram_tensor("f22", [HS, B], F32, kind="ExternalInput")
    wt = nc.dram_tensor("wt", [HS, D_IN], F32R, kind="ExternalInput")
    cutv = nc.dram_tensor("cutv", [128, 1], F32, kind="ExternalInput")
    ftopk = nc.dram_tensor("ftopk", [HS, B], F32R, kind="ExternalOutput")
    recon = nc.dram_tensor("recon", [B, D_IN], F32, kind="ExternalOutput")

    BT2 = 256
    NBT2 = B // BT2  # 16
    NDT = D_IN // 512  # 4 output d-tiles, all accumulated per b-subtile

    with TileContext(nc) as tc:
        with (
            tc.tile_pool(name="wp", bufs=1) as wp,
            tc.tile_pool(name="mp", bufs=1) as mp,
            tc.tile_pool(name="gp", bufs=2) as gp,
            tc.tile_pool(name="cp", bufs=1) as cp,
            tc.tile_pool(name="op", bufs=4) as op,
            tc.tile_pool(name="pp", bufs=2, space="PSUM") as pp,
        ):
            cut_sb = cp.tile([128, 1], F32)
            nc.sync.dma_start(cut_sb, cutv[:, :])
            # W.T shard fully resident: 16 chunks of [128, 2048]
            wt_sb = wp.tile([128, KH * D_IN], F32R, tag="wt")
            for k in range(KH):
                nc.sync.dma_start(
                    wt_sb[:, k * D_IN : (k + 1) * D_IN],
                    wt[k * 128 : (k + 1) * 128, :],
                )
            for bt in range(NBT2):
                m_sb = mp.tile([128, KH * BT2], F32, tag="m")
                for k in range(KH):
                    nc.sync.dma_start(
                        m_sb[:, k * BT2 : (k + 1) * BT2],
                        f22[k * 128 : (k + 1) * 128, bt * BT2 : (bt + 1) * BT2],
                    )
                # masked = (f >= cut) * f, into the f32r staging tile that
                # feeds both the PE and the ftopk output
                g_sb = gp.tile([128, KH * BT2], F32R, tag="g")
                nc.vector.scalar_tensor_tensor(
                    out=g_sb,
                    in0=m_sb,
                    scalar=cut_sb[:, 0:1],
                    in1=m_sb,
                    op0=mybir.AluOpType.is_ge,
                    op1=mybir.AluOpType.mult,
                )
                for k in range(KH):
                    nc.sync.dma_start(
                        ftopk[k * 128 : (k + 1) * 128, bt * BT2 : (bt + 1) * BT2],
                        g_sb[:, k * BT2 : (k + 1) * BT2],
                    )
                for sb in range(BT2 // 128):
                    pts = [
                        pp.tile([128, 512], F32, tag=f"ps{dt}", name=f"pts{dt}")
                        for dt in range(NDT)
                    ]
                    for k in range(KH):
                        lhsT = g_sb[
                            :, k * BT2 + sb * 128 : k * BT2 + (sb + 1) * 128
                        ]
                        for dt in range(NDT):
                            nc.tensor.matmul(
                                pts[dt],
                                lhsT=lhsT,
                                rhs=wt_sb[
                                    :,
                                    k * D_IN + dt * 512 : k * D_IN + (dt + 1) * 512,
                                ],
                                start=(k == 0),
                                stop=(k == KH - 1),
                            )
                    for dt in range(NDT):
                        o_sb = op.tile([128, 512], F32, tag="o", name="o_sb")
                        nc.scalar.copy(o_sb, pts[dt])
                        r0 = bt * BT2 + sb * 128
                        nc.sync.dma_start(
                            recon[r0 : r0 + 128, dt * 512 : (dt + 1) * 512], o_sb
                        )
    if not nc.is_finalized():
        nc.finalize()
    return nc


def _get(name):
    if name not in _CACHE:
        _CACHE[name] = _build_encode() if name == "enc" else _build_decode()
    return _CACHE[name]


def _run_spmd(nc, in_maps):
    trace = bool(os.environ.get("BASS_TRACE"))
    res = run_bass_kernel_spmd(nc, in_maps, core_ids=list(range(NCORES)), trace=trace)
    LAST_RESULTS.append(res)
    return res.results


def kernel(x, W, b_enc, b_dec, k):
    LAST_RESULTS.clear()
    x = np.asarray(x, dtype=np.float32)
    W = np.asarray(W, dtype=np.float32)
    b_enc = np.asarray(b_enc, dtype=np.float32)
    b_dec = np.asarray(b_dec, dtype=np.float32)
    n_keep = int(k) * B

    xT = np.ascontiguousarray(x.T)
    w_shards = [np.ascontiguousarray(W[:, c * HS : (c + 1) * HS]) for c in range(NCORES)]
    wt_shards = [np.ascontiguousarray(ws.T) for ws in w_shards]
    benc_shards = [
        np.ascontiguousarray(
            b_enc[c * HS : (c + 1) * HS].reshape(HS // 128, 128).T
        )
        for c in range(NCORES)
    ]

    # ---- launch 1: encode ----
    enc_in = [
        {"xT": xT, "w": w_shards[c], "benc": benc_shards[c]} for c in range(NCORES)
    ]
    enc_out = _run_spmd(_get("enc"), enc_in)
    f22 = [enc_out[c]["f22"] for c in range(NCORES)]
    samples = np.concatenate([enc_out[c]["samp"].ravel() for c in range(NCORES)])

    # ---- host: conservative threshold from samples ----
    margin = max(8000, n_keep // 32)
    srank = min((n_keep + margin) // SAMPLE_STRIDE, samples.size - 1)
    cut = float(np.partition(samples, samples.size - 1 - srank)[samples.size - 1 - srank])
    cut -= 2e-5
    cut = max(cut, 1e-30)

    for _attempt in range(4):
        out = _decode_and_fix(x, W, b_dec, n_keep, f22, wt_shards, cut)
        if out is not None:
            return out
        cut *= 0.25  # too few survivors; retry with a much lower threshold
    raise RuntimeError("batch top-k threshold selection failed")


def _decode_and_fix(x, W, b_dec, n_keep, f22, wt_shards, cut):
    cut_arr = np.full((128, 1), cut, dtype=np.float32)
    dec_in = [
        {"f22": f22[c], "wt": wt_shards[c], "cutv": cut_arr} for c in range(NCORES)
    ]
    dec_out = _run_spmd(_get("dec"), dec_in)
    # [HS, B] h-major shards (copy: PJRT buffers are read-only)
    ft = [np.array(dec_out[c]["ftopk"]) for c in range(NCORES)]
    recon = dec_out[0]["recon"].astype(np.float32, copy=True)
    for c in range(1, NCORES):
        recon += dec_out[c]["recon"]

    # ---- host: exact top-n_keep boundary fixup ----
    nz_h, nz_b, nz_v, nz_c = [], [], [], []
    for c in range(NCORES):
        hh, bb = np.nonzero(ft[c])
        nz_h.append(hh)
        nz_b.append(bb)
        nz_v.append(ft[c][hh, bb])
        nz_c.append(np.full(hh.shape, c, dtype=np.int32))
    v = np.concatenate(nz_v)
    m = v.size
    if m < n_keep:
        return None
    hl = np.concatenate(nz_h)
    bl = np.concatenate(nz_b)
    cl = np.concatenate(nz_c)
    hg = cl.astype(np.int64) * HS + hl  # global hidden index

    t = float(np.partition(v, m - n_keep)[m - n_keep])  # n-th largest f22 value
    WIN = 1.2e-4  # covers fp22 matmul noise + possible fp22 storage rounding
    if not (cut <= t - WIN):
        return None
    definite = v > t + WIN
    n_def = int(definite.sum())
    bmask = ~definite & (v >= t - WIN)
    n_bnd = int(bmask.sum())
    if n_def > n_keep or n_def + n_bnd < n_keep:
        return None

    # recompute boundary candidates exactly; order like the reference fp32 topk
    bi = np.nonzero(bmask)[0]
    x64 = x.astype(np.float64)
    W64 = W.astype(np.float64)
    exact = np.empty(bi.size, dtype=np.float64)
    for j, i in enumerate(bi):
        exact[j] = np.dot(x64[bl[i]], W64[:, hg[i]])
    exact32 = np.maximum(exact, 0.0).astype(np.float32)
    flat_idx = bl[bi].astype(np.int64) * H + hg[bi]
    order = np.lexsort((flat_idx, -exact32.astype(np.float64)))
    keep_bnd = bi[order[: n_keep - n_def]]

    keep = np.zeros(m, dtype=bool)
    keep[definite] = True
    keep[keep_bnd] = True
    drop = np.nonzero(~keep)[0]

    # zero dropped entries in the f_topk shards
    for c in range(NCORES):
        sel = drop[cl[drop] == c]
        if sel.size:
            ft[c][hl[sel], bl[sel]] = 0.0

    # subtract dropped rank-1 contributions from recon
    if drop.size:
        contrib = v[drop][:, None].astype(np.float32) * W[:, hg[drop]].T
        np.add.at(recon, bl[drop], -contrib)

    recon += b_dec[None, :]

    f_topk = np.empty((B, H), dtype=np.float32)
    for c in range(NCORES):
        f_topk[:, c * HS : (c + 1) * HS] = ft[c].T
    return recon, f_topk


# revision 13
# speedup vs baseline: 1.1318x; 1.0227x over previous
"""BatchTopK Tied SAE kernel for 8 Trainium2 NeuronCores.

Strategy (tensor-parallel over d_hidden):
  - Each core owns a 2048-column shard of W (d_hidden 16384 / 8).
  - Launch 1 (encode): f_shard = relu(x @ W_shard + b_enc_shard), computed
    h-major (fT layout [2048, 4096]) so the decode matmul can consume it
    without any transposes. Stationary operand = W chunks (natural layout),
    moving operand = xT chunks (host-pretransposed x). Single-pass float32r
    (FP22) matmuls. Also emits a strided value sample (every 8th element)
    used by the host to pick a conservative global top-k threshold.
  - Host: picks cut = value at sample-rank (n_keep + margin) so that
    #elements >= cut is guaranteed (whp, verified) to exceed n_keep.
  - Launch 2 (mask + decode): f_masked = f * (f >= cut); writes f_masked
    (h-major) and recon_partial = f_masked @ W_shard.T (fp22 matmuls,
    contraction over the h shard). Host sums the 8 recon partials.
  - Host exactness fixup: the top-n_keep selection boundary is resolved
    exactly on the host: t = n-th largest masked value; elements within a
    small window around t are recomputed in float64 (a few hundred dot
    products) to reproduce the reference's exact fp32 top-k selection; all
    sub-threshold survivors of the conservative cut are zeroed out of
    f_topk and their rank-1 contributions are subtracted from recon.

Self-contained: hardcodes shapes B=4096, D_IN=2048, D_HIDDEN=16384, 8 cores.
"""

import os

import numpy as np

import concourse.bass as bass
import concourse.bacc as bacc
import concourse.mybir as mybir
from concourse.bass_utils import run_bass_kernel_spmd
from concourse.tile import TileContext

F32 = mybir.dt.float32
F32R = mybir.dt.float32r
F16 = mybir.dt.float16
WSCALE = 16.0  # pow2 pre-scale of W in fp16 to avoid subnormal weights

B = 4096
D_IN = 2048
H = 16384
NCORES = 8
HS = H // NCORES  # 2048 hidden columns per core

# encode tiling
KC = D_IN // 128  # 16 contraction chunks
BT = 512  # batch tile (moving free dim)
NBT = B // BT  # 8
NHG = 2  # h-shard halves (W residency)
HTH = HS // NHG // 128  # 8 h-tiles of 128 per half
# decode tiling
NDH = 2  # d_in halves (W.T residency)
DW = D_IN // NDH  # 1024
KH = HS // 128  # 16 contraction chunks over h shard

SAMPLE_STRIDE = 8
NSAMP_COL = B // SAMPLE_STRIDE  # 512

_CACHE = {}

# filled by kernel(): [encode BassKernelResults, decode BassKernelResults]
LAST_RESULTS = []


def _build_encode():
    nc = bacc.Bacc("TRN2")
    xT = nc.dram_tensor("xT", [D_IN, B], F16, kind="ExternalInput")
    w = nc.dram_tensor("w", [D_IN, HS], F16, kind="ExternalInput")
    benc = nc.dram_tensor("benc", [128, HS // 128], F32, kind="ExternalInput")
    f22 = nc.dram_tensor("f22", [HS, B], F32, kind="ExternalOutput")
    samp = nc.dram_tensor("samp", [HS, NSAMP_COL], F32, kind="ExternalOutput")

    NHT = HS // 128  # 16 h-tiles over the full resident W shard

    with TileContext(nc) as tc:
        with (
            tc.tile_pool(name="wp", bufs=1) as wp,
            tc.tile_pool(name="xp", bufs=2) as xp,
            tc.tile_pool(name="fp", bufs=4) as fp,
            tc.tile_pool(name="sp", bufs=4) as sp,
            tc.tile_pool(name="bp", bufs=1) as bp,
            tc.tile_pool(name="pp", bufs=6, space="PSUM") as pp,
        ):
            b_sb = bp.tile([128, HS // 128], F32)
            nc.sync.dma_start(b_sb, benc[:, :])
            # full W shard resident in fp16: chunk k at [:, k*HS:(k+1)*HS]
            w_sb = wp.tile([128, KC * HS], F16, tag="w")
            for k in range(KC):
                nc.sync.dma_start(
                    w_sb[:, k * HS : (k + 1) * HS], w[k * 128 : (k + 1) * 128, :]
                )
            for bt in range(NBT):
                x_sb = xp.tile([128, KC * BT], F16, tag="x")
                for k in range(KC):
                    nc.sync.dma_start(
                        x_sb[:, k * BT : (k + 1) * BT],
                        xT[k * 128 : (k + 1) * 128, bt * BT : (bt + 1) * BT],
                    )
                for ht in range(NHT):
                    ps = pp.tile([128, BT], F32, tag="ps")
                    for k in range(KC):
                        nc.tensor.matmul(
                            ps,
                            lhsT=w_sb[
                                :, k * HS + ht * 128 : k * HS + (ht + 1) * 128
                            ],
                            rhs=x_sb[:, k * BT : (k + 1) * BT],
                            start=(k == 0),
                            stop=(k == KC - 1),
                        )
                    f_sb = fp.tile([128, BT], F32, tag="f")
                    nc.scalar.activation(
                        f_sb,
                        ps,
                        mybir.ActivationFunctionType.Relu,
                        bias=b_sb[:, ht : ht + 1],
                        scale=1.0 / WSCALE,
                    )
                    s_sb = sp.tile([128, BT // SAMPLE_STRIDE], F32, tag="s")
                    nc.vector.tensor_copy(s_sb, f_sb[:, ::SAMPLE_STRIDE])
                    h0 = ht * 128
                    nc.scalar.dma_start(
                        f22[h0 : h0 + 128, bt * BT : (bt + 1) * BT], f_sb
                    )
                    nbs = BT // SAMPLE_STRIDE
                    nc.scalar.dma_start(
                        samp[h0 : h0 + 128, bt * nbs : (bt + 1) * nbs], s_sb
                    )
    if not nc.is_finalized():
        nc.finalize()
    return nc


def _build_decode():
    nc = bacc.Bacc("TRN2")
    f22 = nc.dram_tensor("f22", [HS, B], F32, kind="ExternalInput")
    wt = nc.dram_tensor("wt", [HS, D_IN], F16, kind="ExternalInput")
    cutv = nc.dram_tensor("cutv", [128, 1], F32, kind="ExternalInput")
    ftopk = nc.dram_tensor("ftopk", [HS, B], F32, kind="ExternalOutput")
    recon = nc.dram_tensor("recon", [B, D_IN], F32, kind="ExternalOutput")

    BT2 = 256
    NBT2 = B // BT2  # 16
    NDT = D_IN // 512  # 4 output d-tiles, all accumulated per b-subtile

    with TileContext(nc) as tc:
        with (
            tc.tile_pool(name="wp", bufs=1) as wp,
            tc.tile_pool(name="mp", bufs=2) as mp,
            tc.tile_pool(name="gp", bufs=2) as gp,
            tc.tile_pool(name="cp", bufs=1) as cp,
            tc.tile_pool(name="op", bufs=4) as op,
            tc.tile_pool(name="pp", bufs=2, space="PSUM") as pp,
        ):
            cut_sb = cp.tile([128, 1], F32)
            nc.sync.dma_start(cut_sb, cutv[:, :])
            # W.T shard fully resident in fp16: 16 chunks of [128, 2048]
            wt_sb = wp.tile([128, KH * D_IN], F16, tag="wt")
            for k in range(KH):
                nc.sync.dma_start(
                    wt_sb[:, k * D_IN : (k + 1) * D_IN],
                    wt[k * 128 : (k + 1) * 128, :],
                )
            for bt in range(NBT2):
                m_sb = mp.tile([128, KH * BT2], F32, tag="m")
                for k in range(KH):
                    nc.sync.dma_start(
                        m_sb[:, k * BT2 : (k + 1) * BT2],
                        f22[k * 128 : (k + 1) * 128, bt * BT2 : (bt + 1) * BT2],
                    )
                # masked = (f >= cut) * f in fp32 (ftopk output values), then
                # a fp16 cast copy feeds the PE
                nc.vector.scalar_tensor_tensor(
                    out=m_sb,
                    in0=m_sb,
                    scalar=cut_sb[:, 0:1],
                    in1=m_sb,
                    op0=mybir.AluOpType.is_ge,
                    op1=mybir.AluOpType.mult,
                )
                g_sb = gp.tile([128, KH * BT2], F16, tag="g")
                nc.vector.tensor_copy(g_sb, m_sb)
                for k in range(KH):
                    nc.scalar.dma_start(
                        ftopk[k * 128 : (k + 1) * 128, bt * BT2 : (bt + 1) * BT2],
                        m_sb[:, k * BT2 : (k + 1) * BT2],
                    )
                for sb in range(BT2 // 128):
                    pts = [
                        pp.tile([128, 512], F32, tag=f"ps{dt}", name=f"pts{dt}")
                        for dt in range(NDT)
                    ]
                    for k in range(KH):
                        lhsT = g_sb[
                            :, k * BT2 + sb * 128 : k * BT2 + (sb + 1) * 128
                        ]
                        for dt in range(NDT):
                            nc.tensor.matmul(
                                pts[dt],
                                lhsT=lhsT,
                                rhs=wt_sb[
                                    :,
                                    k * D_IN + dt * 512 : k * D_IN + (dt + 1) * 512,
                                ],
                                start=(k == 0),
                                stop=(k == KH - 1),
                            )
                    for dt in range(NDT):
                        o_sb = op.tile([128, 512], F32, tag="o", name="o_sb")
                        nc.scalar.mul(o_sb, pts[dt], 1.0 / WSCALE)
                        r0 = bt * BT2 + sb * 128
                        nc.scalar.dma_start(
                            recon[r0 : r0 + 128, dt * 512 : (dt + 1) * 512], o_sb
                        )
    if not nc.is_finalized():
        nc.finalize()
    return nc


def _get(name):
    if name not in _CACHE:
        _CACHE[name] = _build_encode() if name == "enc" else _build_decode()
    return _CACHE[name]


def _run_spmd(nc, in_maps):
    trace = bool(os.environ.get("BASS_TRACE"))
    res = run_bass_kernel_spmd(nc, in_maps, core_ids=list(range(NCORES)), trace=trace)
    LAST_RESULTS.append(res)
    return res.results


def kernel(x, W, b_enc, b_dec, k):
    LAST_RESULTS.clear()
    x = np.asarray(x, dtype=np.float32)
    W = np.asarray(W, dtype=np.float32)
    b_enc = np.asarray(b_enc, dtype=np.float32)
    b_dec = np.asarray(b_dec, dtype=np.float32)
    n_keep = int(k) * B

    xT = np.ascontiguousarray(x.T.astype(np.float16))
    w_shards = [
        np.ascontiguousarray((W[:, c * HS : (c + 1) * HS] * WSCALE).astype(np.float16))
        for c in range(NCORES)
    ]
    wt_shards = [np.ascontiguousarray(ws.T) for ws in w_shards]
    benc_shards = [
        np.ascontiguousarray(
            b_enc[c * HS : (c + 1) * HS].reshape(HS // 128, 128).T
        )
        for c in range(NCORES)
    ]

    # ---- launch 1: encode ----
    enc_in = [
        {"xT": xT, "w": w_shards[c], "benc": benc_shards[c]} for c in range(NCORES)
    ]
    enc_out = _run_spmd(_get("enc"), enc_in)
    f22 = [enc_out[c]["f22"] for c in range(NCORES)]
    samples = np.concatenate([enc_out[c]["samp"].ravel() for c in range(NCORES)])

    # ---- host: conservative threshold from samples ----
    margin = max(8000, n_keep // 32)
    srank = min((n_keep + margin) // SAMPLE_STRIDE, samples.size - 1)
    cut = float(np.partition(samples, samples.size - 1 - srank)[samples.size - 1 - srank])
    cut -= 5e-4
    cut = max(cut, 1e-30)

    for _attempt in range(4):
        out = _decode_and_fix(x, W, b_dec, n_keep, f22, wt_shards, cut)
        if out is not None:
            return out
        cut *= 0.25  # too few survivors; retry with a much lower threshold
    raise RuntimeError("batch top-k threshold selection failed")


def _decode_and_fix(x, W, b_dec, n_keep, f22, wt_shards, cut):
    cut_arr = np.full((128, 1), cut, dtype=np.float32)
    dec_in = [
        {"f22": f22[c], "wt": wt_shards[c], "cutv": cut_arr} for c in range(NCORES)
    ]
    dec_out = _run_spmd(_get("dec"), dec_in)
    # [HS, B] h-major shards (copy: PJRT buffers are read-only)
    ft = [np.array(dec_out[c]["ftopk"]) for c in range(NCORES)]
    recon = dec_out[0]["recon"].astype(np.float32, copy=True)
    for c in range(1, NCORES):
        recon += dec_out[c]["recon"]

    # ---- host: exact top-n_keep boundary fixup ----
    nz_h, nz_b, nz_v, nz_c = [], [], [], []
    for c in range(NCORES):
        hh, bb = np.nonzero(ft[c])
        nz_h.append(hh)
        nz_b.append(bb)
        nz_v.append(ft[c][hh, bb])
        nz_c.append(np.full(hh.shape, c, dtype=np.int32))
    v = np.concatenate(nz_v)
    m = v.size
    if m < n_keep:
        return None
    hl = np.concatenate(nz_h)
    bl = np.concatenate(nz_b)
    cl = np.concatenate(nz_c)
    hg = cl.astype(np.int64) * HS + hl  # global hidden index

    t = float(np.partition(v, m - n_keep)[m - n_keep])  # n-th largest f22 value
    WIN = 4e-4  # ~10 sigma of the fp16-input matmul deviation from fp32
    if not (cut <= t - WIN):
        return None
    definite = v > t + WIN
    n_def = int(definite.sum())
    bmask = ~definite & (v >= t - WIN)
    n_bnd = int(bmask.sum())
    if n_def > n_keep or n_def + n_bnd < n_keep:
        return None

    # recompute boundary candidates exactly; order like the reference fp32 topk
    bi = np.nonzero(bmask)[0]
    x64 = x.astype(np.float64)
    W64 = W.astype(np.float64)
    exact = np.empty(bi.size, dtype=np.float64)
    for j, i in enumerate(bi):
        exact[j] = np.dot(x64[bl[i]], W64[:, hg[i]])
    exact32 = np.maximum(exact, 0.0).astype(np.float32)
    flat_idx = bl[bi].astype(np.int64) * H + hg[bi]
    order = np.lexsort((flat_idx, -exact32.astype(np.float64)))
    keep_bnd = bi[order[: n_keep - n_def]]

    keep = np.zeros(m, dtype=bool)
    keep[definite] = True
    keep[keep_bnd] = True
    drop = np.nonzero(~keep)[0]

    # kept boundary entries get the exactly recomputed fp32 values
    exact_vals = np.zeros(m, dtype=np.float32)
    exact_vals[bi] = exact32

    # zero dropped entries / patch kept-boundary entries in the f_topk shards
    for c in range(NCORES):
        sel = drop[cl[drop] == c]
        if sel.size:
            ft[c][hl[sel], bl[sel]] = 0.0
        sel2 = keep_bnd[cl[keep_bnd] == c]
        if sel2.size:
            ft[c][hl[sel2], bl[sel2]] = exact_vals[sel2]

    # subtract dropped rank-1 contributions from recon
    if drop.size:
        contrib = v[drop][:, None].astype(np.float32) * W[:, hg[drop]].T
        np.add.at(recon, bl[drop], -contrib)

    recon += b_dec[None, :]

    f_topk = np.empty((B, H), dtype=np.float32)
    for c in range(NCORES):
        f_topk[:, c * HS : (c + 1) * HS] = ft[c].T
    return recon, f_topk


# revision 14
# speedup vs baseline: 1.2066x; 1.0661x over previous
"""BatchTopK Tied SAE kernel for 8 Trainium2 NeuronCores.

Strategy (tensor-parallel over d_hidden):
  - Each core owns a 2048-column shard of W (d_hidden 16384 / 8).
  - Launch 1 (encode): f_shard = relu(x @ W_shard + b_enc_shard), computed
    h-major (fT layout [2048, 4096]) so the decode matmul can consume it
    without any transposes. Stationary operand = W chunks (natural layout),
    moving operand = xT chunks (host-pretransposed x). Single-pass float32r
    (FP22) matmuls. Also emits a strided value sample (every 8th element)
    used by the host to pick a conservative global top-k threshold.
  - Host: picks cut = value at sample-rank (n_keep + margin) so that
    #elements >= cut is guaranteed (whp, verified) to exceed n_keep.
  - Launch 2 (mask + decode): f_masked = f * (f >= cut); writes f_masked
    (h-major) and recon_partial = f_masked @ W_shard.T (fp22 matmuls,
    contraction over the h shard). Host sums the 8 recon partials.
  - Host exactness fixup: the top-n_keep selection boundary is resolved
    exactly on the host: t = n-th largest masked value; elements within a
    small window around t are recomputed in float64 (a few hundred dot
    products) to reproduce the reference's exact fp32 top-k selection; all
    sub-threshold survivors of the conservative cut are zeroed out of
    f_topk and their rank-1 contributions are subtracted from recon.

Self-contained: hardcodes shapes B=4096, D_IN=2048, D_HIDDEN=16384, 8 cores.
"""

import os

import numpy as np

import concourse.bass as bass
import concourse.bacc as bacc
import concourse.mybir as mybir
from concourse.bass_utils import run_bass_kernel_spmd
from concourse.tile import TileContext

F32 = mybir.dt.float32
F32R = mybir.dt.float32r
F16 = mybir.dt.float16
WSCALE = 16.0  # pow2 pre-scale of W in fp16 to avoid subnormal weights

B = 4096
D_IN = 2048
H = 16384
NCORES = 8
HS = H // NCORES  # 2048 hidden columns per core

# encode tiling
KC = D_IN // 128  # 16 contraction chunks
BT = 512  # batch tile (moving free dim)
NBT = B // BT  # 8
NHG = 2  # h-shard halves (W residency)
HTH = HS // NHG // 128  # 8 h-tiles of 128 per half
# decode tiling
NDH = 2  # d_in halves (W.T residency)
DW = D_IN // NDH  # 1024
KH = HS // 128  # 16 contraction chunks over h shard

SAMPLE_STRIDE = 8
NSAMP_COL = B // SAMPLE_STRIDE  # 512

_CACHE = {}

# filled by kernel(): [encode BassKernelResults, decode BassKernelResults]
LAST_RESULTS = []


def _build_encode():
    nc = bacc.Bacc("TRN2")
    xT = nc.dram_tensor("xT", [D_IN, B], F16, kind="ExternalInput")
    w = nc.dram_tensor("w", [D_IN, HS], F16, kind="ExternalInput")
    benc = nc.dram_tensor("benc", [128, HS // 128], F32, kind="ExternalInput")
    f22 = nc.dram_tensor("f22", [HS, B], F32, kind="ExternalOutput")
    samp = nc.dram_tensor("samp", [HS, NSAMP_COL], F32, kind="ExternalOutput")

    NHT = HS // 128  # 16 h-tiles over the full resident W shard

    with TileContext(nc) as tc:
        with (
            tc.tile_pool(name="wp", bufs=1) as wp,
            tc.tile_pool(name="xp", bufs=2) as xp,
            tc.tile_pool(name="fp", bufs=4) as fp,
            tc.tile_pool(name="sp", bufs=4) as sp,
            tc.tile_pool(name="bp", bufs=1) as bp,
            tc.tile_pool(name="pp", bufs=6, space="PSUM") as pp,
        ):
            b_sb = bp.tile([128, HS // 128], F32)
            nc.sync.dma_start(b_sb, benc[:, :])
            # full W shard resident in fp16: chunk k at [:, k*HS:(k+1)*HS]
            w_sb = wp.tile([128, KC * HS], F16, tag="w")
            for k in range(KC):
                nc.sync.dma_start(
                    w_sb[:, k * HS : (k + 1) * HS], w[k * 128 : (k + 1) * 128, :]
                )
            for bt in range(NBT):
                x_sb = xp.tile([128, KC * BT], F16, tag="x")
                for k in range(KC):
                    nc.sync.dma_start(
                        x_sb[:, k * BT : (k + 1) * BT],
                        xT[k * 128 : (k + 1) * 128, bt * BT : (bt + 1) * BT],
                    )
                for ht in range(NHT):
                    ps = pp.tile([128, BT], F32, tag="ps")
                    for k in range(KC):
                        nc.tensor.matmul(
                            ps,
                            lhsT=w_sb[
                                :, k * HS + ht * 128 : k * HS + (ht + 1) * 128
                            ],
                            rhs=x_sb[:, k * BT : (k + 1) * BT],
                            start=(k == 0),
                            stop=(k == KC - 1),
                        )
                    f_sb = fp.tile([128, BT], F32, tag="f")
                    nc.scalar.activation(
                        f_sb,
                        ps,
                        mybir.ActivationFunctionType.Relu,
                        bias=b_sb[:, ht : ht + 1],
                        scale=1.0 / WSCALE,
                    )
                    s_sb = sp.tile([128, BT // SAMPLE_STRIDE], F32, tag="s")
                    nc.vector.tensor_copy(s_sb, f_sb[:, ::SAMPLE_STRIDE])
                    h0 = ht * 128
                    nc.scalar.dma_start(
                        f22[h0 : h0 + 128, bt * BT : (bt + 1) * BT], f_sb
                    )
                    nbs = BT // SAMPLE_STRIDE
                    nc.scalar.dma_start(
                        samp[h0 : h0 + 128, bt * nbs : (bt + 1) * nbs], s_sb
                    )
    if not nc.is_finalized():
        nc.finalize()
    return nc


def _build_decode():
    nc = bacc.Bacc("TRN2")
    f22 = nc.dram_tensor("f22", [HS, B], F32, kind="ExternalInput")
    wt = nc.dram_tensor("wt", [HS, D_IN], F16, kind="ExternalInput")
    cutv = nc.dram_tensor("cutv", [128, 1], F32, kind="ExternalInput")
    ftopk = nc.dram_tensor("ftopk", [HS, B], F32, kind="ExternalOutput")
    recon = nc.dram_tensor("recon", [B, D_IN], F32, kind="ExternalOutput")

    BT2 = 512
    NBT2 = B // BT2
    NDT = D_IN // 512  # 4 output d-tiles, all accumulated per b-subtile

    with TileContext(nc) as tc:
        with (
            tc.tile_pool(name="wp", bufs=1) as wp,
            tc.tile_pool(name="mp", bufs=2) as mp,
            tc.tile_pool(name="gp", bufs=2) as gp,
            tc.tile_pool(name="cp", bufs=1) as cp,
            tc.tile_pool(name="op", bufs=4) as op,
            tc.tile_pool(name="pp", bufs=2, space="PSUM") as pp,
        ):
            cut_sb = cp.tile([128, 1], F32)
            nc.sync.dma_start(cut_sb, cutv[:, :])
            # W.T shard fully resident in fp16: 16 chunks of [128, 2048]
            wt_sb = wp.tile([128, KH * D_IN], F16, tag="wt")
            for k in range(KH):
                nc.sync.dma_start(
                    wt_sb[:, k * D_IN : (k + 1) * D_IN],
                    wt[k * 128 : (k + 1) * 128, :],
                )
            for bt in range(NBT2):
                m_sb = mp.tile([128, KH * BT2], F32, tag="m")
                for k in range(KH):
                    nc.sync.dma_start(
                        m_sb[:, k * BT2 : (k + 1) * BT2],
                        f22[k * 128 : (k + 1) * 128, bt * BT2 : (bt + 1) * BT2],
                    )
                # masked = (f >= cut) * f in fp32 (ftopk output values), then
                # a fp16 cast copy feeds the PE
                nc.vector.scalar_tensor_tensor(
                    out=m_sb,
                    in0=m_sb,
                    scalar=cut_sb[:, 0:1],
                    in1=m_sb,
                    op0=mybir.AluOpType.is_ge,
                    op1=mybir.AluOpType.mult,
                )
                g_sb = gp.tile([128, KH * BT2], F16, tag="g")
                nc.vector.tensor_copy(g_sb, m_sb)
                for k in range(KH):
                    nc.scalar.dma_start(
                        ftopk[k * 128 : (k + 1) * 128, bt * BT2 : (bt + 1) * BT2],
                        m_sb[:, k * BT2 : (k + 1) * BT2],
                    )
                for sb in range(BT2 // 128):
                    pts = [
                        pp.tile([128, 512], F32, tag=f"ps{dt}", name=f"pts{dt}")
                        for dt in range(NDT)
                    ]
                    for k in range(KH):
                        lhsT = g_sb[
                            :, k * BT2 + sb * 128 : k * BT2 + (sb + 1) * 128
                        ]
                        for dt in range(NDT):
                            nc.tensor.matmul(
                                pts[dt],
                                lhsT=lhsT,
                                rhs=wt_sb[
                                    :,
                                    k * D_IN + dt * 512 : k * D_IN + (dt + 1) * 512,
                                ],
                                start=(k == 0),
                                stop=(k == KH - 1),
                            )
                    for dt in range(NDT):
                        o_sb = op.tile([128, 512], F32, tag="o", name="o_sb")
                        nc.scalar.mul(o_sb, pts[dt], 1.0 / WSCALE)
                        r0 = bt * BT2 + sb * 128
                        nc.scalar.dma_start(
                            recon[r0 : r0 + 128, dt * 512 : (dt + 1) * 512], o_sb
                        )
    if not nc.is_finalized():
        nc.finalize()
    return nc


def _get(name):
    if name not in _CACHE:
        _CACHE[name] = _build_encode() if name == "enc" else _build_decode()
    return _CACHE[name]


def _run_spmd(nc, in_maps):
    trace = bool(os.environ.get("BASS_TRACE"))
    res = run_bass_kernel_spmd(nc, in_maps, core_ids=list(range(NCORES)), trace=trace)
    LAST_RESULTS.append(res)
    return res.results


def kernel(x, W, b_enc, b_dec, k):
    LAST_RESULTS.clear()
    x = np.asarray(x, dtype=np.float32)
    W = np.asarray(W, dtype=np.float32)
    b_enc = np.asarray(b_enc, dtype=np.float32)
    b_dec = np.asarray(b_dec, dtype=np.float32)
    n_keep = int(k) * B

    xT = np.ascontiguousarray(x.T.astype(np.float16))
    w_shards = [
        np.ascontiguousarray((W[:, c * HS : (c + 1) * HS] * WSCALE).astype(np.float16))
        for c in range(NCORES)
    ]
    wt_shards = [np.ascontiguousarray(ws.T) for ws in w_shards]
    benc_shards = [
        np.ascontiguousarray(
            b_enc[c * HS : (c + 1) * HS].reshape(HS // 128, 128).T
        )
        for c in range(NCORES)
    ]

    # ---- launch 1: encode ----
    enc_in = [
        {"xT": xT, "w": w_shards[c], "benc": benc_shards[c]} for c in range(NCORES)
    ]
    enc_out = _run_spmd(_get("enc"), enc_in)
    f22 = [enc_out[c]["f22"] for c in range(NCORES)]
    samples = np.concatenate([enc_out[c]["samp"].ravel() for c in range(NCORES)])

    # ---- host: conservative threshold from samples ----
    margin = max(8000, n_keep // 32)
    srank = min((n_keep + margin) // SAMPLE_STRIDE, samples.size - 1)
    cut = float(np.partition(samples, samples.size - 1 - srank)[samples.size - 1 - srank])
    cut -= 5e-4
    cut = max(cut, 1e-30)

    for _attempt in range(4):
        out = _decode_and_fix(x, W, b_dec, n_keep, f22, wt_shards, cut)
        if out is not None:
            return out
        cut *= 0.25  # too few survivors; retry with a much lower threshold
    raise RuntimeError("batch top-k threshold selection failed")


def _decode_and_fix(x, W, b_dec, n_keep, f22, wt_shards, cut):
    cut_arr = np.full((128, 1), cut, dtype=np.float32)
    dec_in = [
        {"f22": f22[c], "wt": wt_shards[c], "cutv": cut_arr} for c in range(NCORES)
    ]
    dec_out = _run_spmd(_get("dec"), dec_in)
    # [HS, B] h-major shards (copy: PJRT buffers are read-only)
    ft = [np.array(dec_out[c]["ftopk"]) for c in range(NCORES)]
    recon = dec_out[0]["recon"].astype(np.float32, copy=True)
    for c in range(1, NCORES):
        recon += dec_out[c]["recon"]

    # ---- host: exact top-n_keep boundary fixup ----
    nz_h, nz_b, nz_v, nz_c = [], [], [], []
    for c in range(NCORES):
        hh, bb = np.nonzero(ft[c])
        nz_h.append(hh)
        nz_b.append(bb)
        nz_v.append(ft[c][hh, bb])
        nz_c.append(np.full(hh.shape, c, dtype=np.int32))
    v = np.concatenate(nz_v)
    m = v.size
    if m < n_keep:
        return None
    hl = np.concatenate(nz_h)
    bl = np.concatenate(nz_b)
    cl = np.concatenate(nz_c)
    hg = cl.astype(np.int64) * HS + hl  # global hidden index

    t = float(np.partition(v, m - n_keep)[m - n_keep])  # n-th largest f22 value
    WIN = 4e-4  # ~10 sigma of the fp16-input matmul deviation from fp32
    if not (cut <= t - WIN):
        return None
    definite = v > t + WIN
    n_def = int(definite.sum())
    bmask = ~definite & (v >= t - WIN)
    n_bnd = int(bmask.sum())
    if n_def > n_keep or n_def + n_bnd < n_keep:
        return None

    # recompute boundary candidates exactly; order like the reference fp32 topk
    bi = np.nonzero(bmask)[0]
    x64 = x.astype(np.float64)
    W64 = W.astype(np.float64)
    exact = np.empty(bi.size, dtype=np.float64)
    for j, i in enumerate(bi):
        exact[j] = np.dot(x64[bl[i]], W64[:, hg[i]])
    exact32 = np.maximum(exact, 0.0).astype(np.float32)
    flat_idx = bl[bi].astype(np.int64) * H + hg[bi]
    order = np.lexsort((flat_idx, -exact32.astype(np.float64)))
    keep_bnd = bi[order[: n_keep - n_def]]

    keep = np.zeros(m, dtype=bool)
    keep[definite] = True
    keep[keep_bnd] = True
    drop = np.nonzero(~keep)[0]

    # kept boundary entries get the exactly recomputed fp32 values
    exact_vals = np.zeros(m, dtype=np.float32)
    exact_vals[bi] = exact32

    # zero dropped entries / patch kept-boundary entries in the f_topk shards
    for c in range(NCORES):
        sel = drop[cl[drop] == c]
        if sel.size:
            ft[c][hl[sel], bl[sel]] = 0.0
        sel2 = keep_bnd[cl[keep_bnd] == c]
        if sel2.size:
            ft[c][hl[sel2], bl[sel2]] = exact_vals[sel2]

    # subtract dropped rank-1 contributions from recon
    if drop.size:
        contrib = v[drop][:, None].astype(np.float32) * W[:, hg[drop]].T
        np.add.at(recon, bl[drop], -contrib)

    recon += b_dec[None, :]

    f_topk = np.empty((B, H), dtype=np.float32)
    for c in range(NCORES):
        f_topk[:, c * HS : (c + 1) * HS] = ft[c].T
    return recon, f_topk


# revision 15
# speedup vs baseline: 1.2293x; 1.0188x over previous
"""BatchTopK Tied SAE kernel for 8 Trainium2 NeuronCores.

Strategy (tensor-parallel over d_hidden):
  - Each core owns a 2048-column shard of W (d_hidden 16384 / 8).
  - Launch 1 (encode): f_shard = relu(x @ W_shard + b_enc_shard), computed
    h-major (fT layout [2048, 4096]) so the decode matmul can consume it
    without any transposes. Stationary operand = W chunks (natural layout),
    moving operand = xT chunks (host-pretransposed x). Single-pass float32r
    (FP22) matmuls. Also emits a strided value sample (every 8th element)
    used by the host to pick a conservative global top-k threshold.
  - Host: picks cut = value at sample-rank (n_keep + margin) so that
    #elements >= cut is guaranteed (whp, verified) to exceed n_keep.
  - Launch 2 (mask + decode): f_masked = f * (f >= cut); writes f_masked
    (h-major) and recon_partial = f_masked @ W_shard.T (fp22 matmuls,
    contraction over the h shard). Host sums the 8 recon partials.
  - Host exactness fixup: the top-n_keep selection boundary is resolved
    exactly on the host: t = n-th largest masked value; elements within a
    small window around t are recomputed in float64 (a few hundred dot
    products) to reproduce the reference's exact fp32 top-k selection; all
    sub-threshold survivors of the conservative cut are zeroed out of
    f_topk and their rank-1 contributions are subtracted from recon.

Self-contained: hardcodes shapes B=4096, D_IN=2048, D_HIDDEN=16384, 8 cores.
"""

import os

import numpy as np

import concourse.bass as bass
import concourse.bacc as bacc
import concourse.mybir as mybir
from concourse.bass_utils import run_bass_kernel_spmd
from concourse.tile import TileContext

F32 = mybir.dt.float32
F32R = mybir.dt.float32r
F16 = mybir.dt.float16
WSCALE = 16.0  # pow2 pre-scale of W in fp16 to avoid subnormal weights

B = 4096
D_IN = 2048
H = 16384
NCORES = 8
HS = H // NCORES  # 2048 hidden columns per core

# encode tiling
KC = D_IN // 128  # 16 contraction chunks
BT = 512  # batch tile (moving free dim)
NBT = B // BT  # 8
NHG = 2  # h-shard halves (W residency)
HTH = HS // NHG // 128  # 8 h-tiles of 128 per half
# decode tiling
NDH = 2  # d_in halves (W.T residency)
DW = D_IN // NDH  # 1024
KH = HS // 128  # 16 contraction chunks over h shard

SAMPLE_STRIDE = 8
NSAMP_COL = B // SAMPLE_STRIDE  # 512

_CACHE = {}

# filled by kernel(): [encode BassKernelResults, decode BassKernelResults]
LAST_RESULTS = []


def _build_encode():
    nc = bacc.Bacc("TRN2")
    xT = nc.dram_tensor("xT", [D_IN, B], F16, kind="ExternalInput")
    w = nc.dram_tensor("w", [D_IN, HS], F16, kind="ExternalInput")
    benc = nc.dram_tensor("benc", [128, HS // 128], F32, kind="ExternalInput")
    f22 = nc.dram_tensor("f22", [HS, B], F32, kind="ExternalOutput")
    samp = nc.dram_tensor("samp", [HS, NSAMP_COL], F32, kind="ExternalOutput")

    NHT = HS // 128  # 16 h-tiles over the full resident W shard
    NQ = 4  # x quarter-tiles (4 contraction chunks each) for chunk-level deps

    with TileContext(nc) as tc:
        with (
            tc.tile_pool(name="wp", bufs=1) as wp,
            tc.tile_pool(name="xp", bufs=2) as xp,
            tc.tile_pool(name="fp", bufs=4) as fp,
            tc.tile_pool(name="sp", bufs=4) as sp,
            tc.tile_pool(name="bp", bufs=1) as bp,
            tc.tile_pool(name="pp", bufs=6, space="PSUM") as pp,
        ):
            b_sb = bp.tile([128, HS // 128], F32)
            nc.sync.dma_start(b_sb, benc[:, :])

            def load_x(bt):
                qs = []
                for q in range(NQ):
                    xq = xp.tile([128, 4 * BT], F16, tag=f"x{q}", name=f"x{q}")
                    for j in range(4):
                        k = q * 4 + j
                        nc.sync.dma_start(
                            xq[:, j * BT : (j + 1) * BT],
                            xT[k * 128 : (k + 1) * 128, bt * BT : (bt + 1) * BT],
                        )
                    qs.append(xq)
                return qs

            # stage bt0's x before the resident W so the PE can start on
            # chunk 0 while the rest of W is still loading
            x0 = load_x(0)
            w_sbs = []
            for k in range(KC):
                wk = wp.tile([128, HS], F16, tag=f"w{k}", name=f"w{k}")
                nc.sync.dma_start(wk, w[k * 128 : (k + 1) * 128, :])
                w_sbs.append(wk)
            for bt in range(NBT):
                xqs = x0 if bt == 0 else load_x(bt)
                for ht in range(NHT):
                    ps = pp.tile([128, BT], F32, tag="ps")
                    for k in range(KC):
                        nc.tensor.matmul(
                            ps,
                            lhsT=w_sbs[k][:, ht * 128 : (ht + 1) * 128],
                            rhs=xqs[k // 4][:, (k % 4) * BT : (k % 4 + 1) * BT],
                            start=(k == 0),
                            stop=(k == KC - 1),
                        )
                    f_sb = fp.tile([128, BT], F32, tag="f")
                    nc.scalar.activation(
                        f_sb,
                        ps,
                        mybir.ActivationFunctionType.Relu,
                        bias=b_sb[:, ht : ht + 1],
                        scale=1.0 / WSCALE,
                    )
                    s_sb = sp.tile([128, BT // SAMPLE_STRIDE], F32, tag="s")
                    nc.vector.tensor_copy(s_sb, f_sb[:, ::SAMPLE_STRIDE])
                    h0 = ht * 128
                    nc.scalar.dma_start(
                        f22[h0 : h0 + 128, bt * BT : (bt + 1) * BT], f_sb
                    )
                    nbs = BT // SAMPLE_STRIDE
                    nc.scalar.dma_start(
                        samp[h0 : h0 + 128, bt * nbs : (bt + 1) * nbs], s_sb
                    )
    if not nc.is_finalized():
        nc.finalize()
    return nc


def _build_decode():
    nc = bacc.Bacc("TRN2")
    f22 = nc.dram_tensor("f22", [HS, B], F32, kind="ExternalInput")
    wt = nc.dram_tensor("wt", [HS, D_IN], F16, kind="ExternalInput")
    cutv = nc.dram_tensor("cutv", [128, 1], F32, kind="ExternalInput")
    ftopk = nc.dram_tensor("ftopk", [HS, B], F32, kind="ExternalOutput")
    recon = nc.dram_tensor("recon", [B, D_IN], F32, kind="ExternalOutput")

    BT2 = 512
    NBT2 = B // BT2  # 8
    NDT = D_IN // 512  # 4 output d-tiles, all accumulated per b-subtile
    NQ = 4  # mask pipeline quarters (4 contraction chunks each)

    with TileContext(nc) as tc:
        with (
            tc.tile_pool(name="wp", bufs=1) as wp,
            tc.tile_pool(name="mp", bufs=2) as mp,
            tc.tile_pool(name="gp", bufs=2) as gp,
            tc.tile_pool(name="cp", bufs=1) as cp,
            tc.tile_pool(name="op", bufs=4) as op,
            tc.tile_pool(name="pp", bufs=2, space="PSUM") as pp,
        ):
            cut_sb = cp.tile([128, 1], F32)
            nc.sync.dma_start(cut_sb, cutv[:, :])

            def stage_bt(bt):
                gs = []
                for q in range(NQ):
                    mq = mp.tile([128, 4 * BT2], F32, tag=f"m{q}", name=f"m{q}")
                    for j in range(4):
                        k = q * 4 + j
                        nc.sync.dma_start(
                            mq[:, j * BT2 : (j + 1) * BT2],
                            f22[k * 128 : (k + 1) * 128, bt * BT2 : (bt + 1) * BT2],
                        )
                    # masked = (f >= cut) * f in fp32 (ftopk values), then a
                    # fp16 cast copy feeds the PE
                    nc.vector.scalar_tensor_tensor(
                        out=mq,
                        in0=mq,
                        scalar=cut_sb[:, 0:1],
                        in1=mq,
                        op0=mybir.AluOpType.is_ge,
                        op1=mybir.AluOpType.mult,
                    )
                    gq = gp.tile([128, 4 * BT2], F16, tag=f"g{q}", name=f"g{q}")
                    nc.vector.tensor_copy(gq, mq)
                    for j in range(4):
                        k = q * 4 + j
                        nc.scalar.dma_start(
                            ftopk[k * 128 : (k + 1) * 128, bt * BT2 : (bt + 1) * BT2],
                            mq[:, j * BT2 : (j + 1) * BT2],
                        )
                    gs.append(gq)
                return gs

            # stage bt0's mask pipeline ahead of the resident W.T loads
            g0 = stage_bt(0)
            wt_sbs = []
            for k in range(KH):
                wk = wp.tile([128, D_IN], F16, tag=f"wt{k}", name=f"wt{k}")
                nc.sync.dma_start(wk, wt[k * 128 : (k + 1) * 128, :])
                wt_sbs.append(wk)
            for bt in range(NBT2):
                gs = g0 if bt == 0 else stage_bt(bt)
                for sb in range(BT2 // 128):
                    pts = [
                        pp.tile([128, 512], F32, tag=f"ps{dt}", name=f"pts{dt}")
                        for dt in range(NDT)
                    ]
                    for k in range(KH):
                        lhsT = gs[k // 4][
                            :,
                            (k % 4) * BT2 + sb * 128 : (k % 4) * BT2 + (sb + 1) * 128,
                        ]
                        for dt in range(NDT):
                            nc.tensor.matmul(
                                pts[dt],
                                lhsT=lhsT,
                                rhs=wt_sbs[k][:, dt * 512 : (dt + 1) * 512],
                                start=(k == 0),
                                stop=(k == KH - 1),
                            )
                    for dt in range(NDT):
                        o_sb = op.tile([128, 512], F32, tag="o", name="o_sb")
                        nc.scalar.mul(o_sb, pts[dt], 1.0 / WSCALE)
                        r0 = bt * BT2 + sb * 128
                        nc.scalar.dma_start(
                            recon[r0 : r0 + 128, dt * 512 : (dt + 1) * 512], o_sb
                        )
    if not nc.is_finalized():
        nc.finalize()
    return nc


def _get(name):
    if name not in _CACHE:
        _CACHE[name] = _build_encode() if name == "enc" else _build_decode()
    return _CACHE[name]


def _run_spmd(nc, in_maps):
    trace = bool(os.environ.get("BASS_TRACE"))
    res = run_bass_kernel_spmd(nc, in_maps, core_ids=list(range(NCORES)), trace=trace)
    LAST_RESULTS.append(res)
    return res.results


def kernel(x, W, b_enc, b_dec, k):
    LAST_RESULTS.clear()
    x = np.asarray(x, dtype=np.float32)
    W = np.asarray(W, dtype=np.float32)
    b_enc = np.asarray(b_enc, dtype=np.float32)
    b_dec = np.asarray(b_dec, dtype=np.float32)
    n_keep = int(k) * B

    xT = np.ascontiguousarray(x.T.astype(np.float16))
    w_shards = [
        np.ascontiguousarray((W[:, c * HS : (c + 1) * HS] * WSCALE).astype(np.float16))
        for c in range(NCORES)
    ]
    wt_shards = [np.ascontiguousarray(ws.T) for ws in w_shards]
    benc_shards = [
        np.ascontiguousarray(
            b_enc[c * HS : (c + 1) * HS].reshape(HS // 128, 128).T
        )
        for c in range(NCORES)
    ]

    # ---- launch 1: encode ----
    enc_in = [
        {"xT": xT, "w": w_shards[c], "benc": benc_shards[c]} for c in range(NCORES)
    ]
    enc_out = _run_spmd(_get("enc"), enc_in)
    f22 = [enc_out[c]["f22"] for c in range(NCORES)]
    samples = np.concatenate([enc_out[c]["samp"].ravel() for c in range(NCORES)])

    # ---- host: conservative threshold from samples ----
    margin = max(8000, n_keep // 32)
    srank = min((n_keep + margin) // SAMPLE_STRIDE, samples.size - 1)
    cut = float(np.partition(samples, samples.size - 1 - srank)[samples.size - 1 - srank])
    cut -= 5e-4
    cut = max(cut, 1e-30)

    for _attempt in range(4):
        out = _decode_and_fix(x, W, b_dec, n_keep, f22, wt_shards, cut)
        if out is not None:
            return out
        cut *= 0.25  # too few survivors; retry with a much lower threshold
    raise RuntimeError("batch top-k threshold selection failed")


def _decode_and_fix(x, W, b_dec, n_keep, f22, wt_shards, cut):
    cut_arr = np.full((128, 1), cut, dtype=np.float32)
    dec_in = [
        {"f22": f22[c], "wt": wt_shards[c], "cutv": cut_arr} for c in range(NCORES)
    ]
    dec_out = _run_spmd(_get("dec"), dec_in)
    # [HS, B] h-major shards (copy: PJRT buffers are read-only)
    ft = [np.array(dec_out[c]["ftopk"]) for c in range(NCORES)]
    recon = dec_out[0]["recon"].astype(np.float32, copy=True)
    for c in range(1, NCORES):
        recon += dec_out[c]["recon"]

    # ---- host: exact top-n_keep boundary fixup ----
    nz_h, nz_b, nz_v, nz_c = [], [], [], []
    for c in range(NCORES):
        hh, bb = np.nonzero(ft[c])
        nz_h.append(hh)
        nz_b.append(bb)
        nz_v.append(ft[c][hh, bb])
        nz_c.append(np.full(hh.shape, c, dtype=np.int32))
    v = np.concatenate(nz_v)
    m = v.size
    if m < n_keep:
        return None
    hl = np.concatenate(nz_h)
    bl = np.concatenate(nz_b)
    cl = np.concatenate(nz_c)
    hg = cl.astype(np.int64) * HS + hl  # global hidden index

    t = float(np.partition(v, m - n_keep)[m - n_keep])  # n-th largest f22 value
    WIN = 4e-4  # ~10 sigma of the fp16-input matmul deviation from fp32
    if not (cut <= t - WIN):
        return None
    definite = v > t + WIN
    n_def = int(definite.sum())
    bmask = ~definite & (v >= t - WIN)
    n_bnd = int(bmask.sum())
    if n_def > n_keep or n_def + n_bnd < n_keep:
        return None

    # recompute boundary candidates exactly; order like the reference fp32 topk
    bi = np.nonzero(bmask)[0]
    x64 = x.astype(np.float64)
    W64 = W.astype(np.float64)
    exact = np.empty(bi.size, dtype=np.float64)
    for j, i in enumerate(bi):
        exact[j] = np.dot(x64[bl[i]], W64[:, hg[i]])
    exact32 = np.maximum(exact, 0.0).astype(np.float32)
    flat_idx = bl[bi].astype(np.int64) * H + hg[bi]
    order = np.lexsort((flat_idx, -exact32.astype(np.float64)))
    keep_bnd = bi[order[: n_keep - n_def]]

    keep = np.zeros(m, dtype=bool)
    keep[definite] = True
    keep[keep_bnd] = True
    drop = np.nonzero(~keep)[0]

    # kept boundary entries get the exactly recomputed fp32 values
    exact_vals = np.zeros(m, dtype=np.float32)
    exact_vals[bi] = exact32

    # zero dropped entries / patch kept-boundary entries in the f_topk shards
    for c in range(NCORES):
        sel = drop[cl[drop] == c]
        if sel.size:
            ft[c][hl[sel], bl[sel]] = 0.0
        sel2 = keep_bnd[cl[keep_bnd] == c]
        if sel2.size:
            ft[c][hl[sel2], bl[sel2]] = exact_vals[sel2]

    # subtract dropped rank-1 contributions from recon
    if drop.size:
        contrib = v[drop][:, None].astype(np.float32) * W[:, hg[drop]].T
        np.add.at(recon, bl[drop], -contrib)

    recon += b_dec[None, :]

    f_topk = np.empty((B, H), dtype=np.float32)
    for c in range(NCORES):
        f_topk[:, c * HS : (c + 1) * HS] = ft[c].T
    return recon, f_topk


# revision 17
# speedup vs baseline: 1.2526x; 1.0189x over previous
"""BatchTopK Tied SAE kernel for 8 Trainium2 NeuronCores.

Strategy (tensor-parallel over d_hidden):
  - Each core owns a 2048-column shard of W (d_hidden 16384 / 8).
  - Launch 1 (encode): f_shard = relu(x @ W_shard + b_enc_shard), computed
    h-major (fT layout [2048, 4096]) so the decode matmul can consume it
    without any transposes. Stationary operand = W chunks (natural layout),
    moving operand = xT chunks (host-pretransposed x). Matmuls run in fp16
    (PE-internal FP22), with W pre-scaled by 16 to avoid fp16 subnormals. Also emits a strided value sample (every 8th element)
    used by the host to pick a conservative global top-k threshold.
  - Host: picks cut = value at sample-rank (n_keep + margin) so that
    #elements >= cut is guaranteed (whp, verified) to exceed n_keep.
  - Launch 2 (mask + decode): f_masked = f * (f >= cut); writes f_masked
    (h-major) and recon_partial = f_masked @ W_shard.T (fp16 matmuls,
    contraction over the h shard). Host sums the 8 recon partials.
  - Host exactness fixup: the top-n_keep selection boundary is resolved
    exactly on the host: t = n-th largest masked value; elements within a
    small window around t are recomputed in float64 (a few hundred dot
    products) to reproduce the reference's exact fp32 top-k selection; all
    sub-threshold survivors of the conservative cut are zeroed out of
    f_topk and their rank-1 contributions are subtracted from recon.

Self-contained: hardcodes shapes B=4096, D_IN=2048, D_HIDDEN=16384, 8 cores.
"""

import os

import numpy as np

import concourse.bass as bass
import concourse.bacc as bacc
import concourse.mybir as mybir
from concourse.bass_utils import run_bass_kernel_spmd
from concourse.tile import TileContext

F32 = mybir.dt.float32
F32R = mybir.dt.float32r
F16 = mybir.dt.float16
WSCALE = 16.0  # pow2 pre-scale of W in fp16 to avoid subnormal weights

B = 4096
D_IN = 2048
H = 16384
NCORES = 8
HS = H // NCORES  # 2048 hidden columns per core

# encode tiling
KC = D_IN // 128  # 16 contraction chunks
BT = 512  # batch tile (moving free dim)
NBT = B // BT  # 8
NHG = 2  # h-shard halves (W residency)
HTH = HS // NHG // 128  # 8 h-tiles of 128 per half
# decode tiling
NDH = 2  # d_in halves (W.T residency)
DW = D_IN // NDH  # 1024
KH = HS // 128  # 16 contraction chunks over h shard

SAMPLE_STRIDE = 8
NSAMP_COL = B // SAMPLE_STRIDE  # 512

_CACHE = {}

# filled by kernel(): [encode BassKernelResults, decode BassKernelResults]
LAST_RESULTS = []


def _build_encode():
    nc = bacc.Bacc("TRN2")
    xT = nc.dram_tensor("xT", [D_IN, B], F16, kind="ExternalInput")
    w = nc.dram_tensor("w", [D_IN, HS], F16, kind="ExternalInput")
    benc = nc.dram_tensor("benc", [128, HS // 128], F32, kind="ExternalInput")
    f22 = nc.dram_tensor("f22", [HS, B], F32, kind="ExternalOutput")
    samp = nc.dram_tensor("samp", [HS, NSAMP_COL], F32, kind="ExternalOutput")

    NHT = HS // 128  # 16 h-tiles over the full resident W shard
    NQ = 4  # x quarter-tiles (4 contraction chunks each) for chunk-level deps

    with TileContext(nc) as tc:
        with (
            tc.tile_pool(name="wp", bufs=1) as wp,
            tc.tile_pool(name="xp", bufs=2) as xp,
            tc.tile_pool(name="fp", bufs=4) as fp,
            tc.tile_pool(name="sp", bufs=4) as sp,
            tc.tile_pool(name="bp", bufs=1) as bp,
            tc.tile_pool(name="pp", bufs=6, space="PSUM") as pp,
        ):
            b_sb = bp.tile([128, HS // 128], F32)
            nc.sync.dma_start(b_sb, benc[:, :])

            def load_x(bt):
                qs = []
                for q in range(NQ):
                    xq = xp.tile([128, 4 * BT], F16, tag=f"x{q}", name=f"x{q}")
                    for j in range(4):
                        k = q * 4 + j
                        nc.sync.dma_start(
                            xq[:, j * BT : (j + 1) * BT],
                            xT[k * 128 : (k + 1) * 128, bt * BT : (bt + 1) * BT],
                        )
                    qs.append(xq)
                return qs

            # stage bt0's x before the resident W so the PE can start on
            # chunk 0 while the rest of W is still loading
            x0 = load_x(0)
            w_sbs = []
            for k in range(KC):
                wk = wp.tile([128, HS], F16, tag=f"w{k}", name=f"w{k}")
                nc.sync.dma_start(wk, w[k * 128 : (k + 1) * 128, :])
                w_sbs.append(wk)
            for bt in range(NBT):
                xqs = x0 if bt == 0 else load_x(bt)
                for ht in range(NHT):
                    ps = pp.tile([128, BT], F32, tag="ps")
                    for k in range(KC):
                        nc.tensor.matmul(
                            ps,
                            lhsT=w_sbs[k][:, ht * 128 : (ht + 1) * 128],
                            rhs=xqs[k // 4][:, (k % 4) * BT : (k % 4 + 1) * BT],
                            start=(k == 0),
                            stop=(k == KC - 1),
                        )
                    f_sb = fp.tile([128, BT], F32, tag="f")
                    nc.scalar.activation(
                        f_sb,
                        ps,
                        mybir.ActivationFunctionType.Relu,
                        bias=b_sb[:, ht : ht + 1],
                        scale=1.0 / WSCALE,
                    )
                    s_sb = sp.tile([128, BT // SAMPLE_STRIDE], F32, tag="s")
                    nc.vector.tensor_copy(s_sb, f_sb[:, ::SAMPLE_STRIDE])
                    h0 = ht * 128
                    nc.scalar.dma_start(
                        f22[h0 : h0 + 128, bt * BT : (bt + 1) * BT], f_sb
                    )
                    nbs = BT // SAMPLE_STRIDE
                    nc.scalar.dma_start(
                        samp[h0 : h0 + 128, bt * nbs : (bt + 1) * nbs], s_sb
                    )
    if not nc.is_finalized():
        nc.finalize()
    return nc


def _build_decode():
    nc = bacc.Bacc("TRN2")
    f22 = nc.dram_tensor("f22", [HS, B], F32, kind="ExternalInput")
    wt = nc.dram_tensor("wt", [HS, D_IN], F16, kind="ExternalInput")
    cutv = nc.dram_tensor("cutv", [128, 1], F32, kind="ExternalInput")
    ftopk = nc.dram_tensor("ftopk", [HS, B], F32, kind="ExternalOutput")
    recon = nc.dram_tensor("recon", [B, D_IN], F32, kind="ExternalOutput")

    BT2 = 512
    NBT2 = B // BT2  # 8
    NDT = D_IN // 512  # 4 output d-tiles, all accumulated per b-subtile
    NQ = 4  # mask pipeline quarters (4 contraction chunks each)

    with TileContext(nc) as tc:
        with (
            tc.tile_pool(name="wp", bufs=1) as wp,
            tc.tile_pool(name="mp", bufs=2) as mp,
            tc.tile_pool(name="gp", bufs=2) as gp,
            tc.tile_pool(name="cp", bufs=1) as cp,
            tc.tile_pool(name="op", bufs=4) as op,
            tc.tile_pool(name="pp", bufs=2, space="PSUM") as pp,
        ):
            cut_sb = cp.tile([128, 1], F32)
            nc.sync.dma_start(cut_sb, cutv[:, :])

            wt_sbs = [None] * KH

            def load_wt(k):
                wk = wp.tile([128, D_IN], F16, tag=f"wt{k}", name=f"wt{k}")
                nc.sync.dma_start(wk, wt[k * 128 : (k + 1) * 128, :])
                wt_sbs[k] = wk

            def stage_bt(bt, wt_interleave=False):
                gs = []
                for q in range(NQ):
                    mq = mp.tile([128, 4 * BT2], F32, tag=f"m{q}", name=f"m{q}")
                    for j in range(4):
                        k = q * 4 + j
                        nc.sync.dma_start(
                            mq[:, j * BT2 : (j + 1) * BT2],
                            f22[k * 128 : (k + 1) * 128, bt * BT2 : (bt + 1) * BT2],
                        )
                    # masked = (f >= cut) * f in fp32 (ftopk values), then a
                    # fp16 cast copy feeds the PE
                    nc.vector.scalar_tensor_tensor(
                        out=mq,
                        in0=mq,
                        scalar=cut_sb[:, 0:1],
                        in1=mq,
                        op0=mybir.AluOpType.is_ge,
                        op1=mybir.AluOpType.mult,
                    )
                    gq = gp.tile([128, 4 * BT2], F16, tag=f"g{q}", name=f"g{q}")
                    nc.scalar.copy(gq, mq)
                    for j in range(4):
                        k = q * 4 + j
                        nc.scalar.dma_start(
                            ftopk[k * 128 : (k + 1) * 128, bt * BT2 : (bt + 1) * BT2],
                            mq[:, j * BT2 : (j + 1) * BT2],
                        )
                    gs.append(gq)
                    if wt_interleave:
                        for k in range(q * 4, (q + 1) * 4):
                            load_wt(k)
                return gs

            # stage bt0's mask pipeline with the resident W.T chunk loads
            # interleaved, so the first matmuls have both operands early
            g0 = stage_bt(0, wt_interleave=True)
            for bt in range(NBT2):
                gs = g0 if bt == 0 else stage_bt(bt)
                for sb in range(BT2 // 128):
                    pts = [
                        pp.tile([128, 512], F32, tag=f"ps{dt}", name=f"pts{dt}")
                        for dt in range(NDT)
                    ]
                    for k in range(KH):
                        lhsT = gs[k // 4][
                            :,
                            (k % 4) * BT2 + sb * 128 : (k % 4) * BT2 + (sb + 1) * 128,
                        ]
                        for dt in range(NDT):
                            nc.tensor.matmul(
                                pts[dt],
                                lhsT=lhsT,
                                rhs=wt_sbs[k][:, dt * 512 : (dt + 1) * 512],
                                start=(k == 0),
                                stop=(k == KH - 1),
                            )
                    for dt in range(NDT):
                        o_sb = op.tile([128, 512], F32, tag="o", name="o_sb")
                        nc.scalar.mul(o_sb, pts[dt], 1.0 / WSCALE)
                        r0 = bt * BT2 + sb * 128
                        nc.scalar.dma_start(
                            recon[r0 : r0 + 128, dt * 512 : (dt + 1) * 512], o_sb
                        )
    if not nc.is_finalized():
        nc.finalize()
    return nc


def _get(name):
    if name not in _CACHE:
        _CACHE[name] = _build_encode() if name == "enc" else _build_decode()
    return _CACHE[name]


def _run_spmd(nc, in_maps):
    trace = bool(os.environ.get("BASS_TRACE"))
    res = run_bass_kernel_spmd(nc, in_maps, core_ids=list(range(NCORES)), trace=trace)
    LAST_RESULTS.append(res)
    return res.results


def kernel(x, W, b_enc, b_dec, k):
    LAST_RESULTS.clear()
    x = np.asarray(x, dtype=np.float32)
    W = np.asarray(W, dtype=np.float32)
    b_enc = np.asarray(b_enc, dtype=np.float32)
    b_dec = np.asarray(b_dec, dtype=np.float32)
    n_keep = int(k) * B

    xT = np.ascontiguousarray(x.T.astype(np.float16))
    w_shards = [
        np.ascontiguousarray((W[:, c * HS : (c + 1) * HS] * WSCALE).astype(np.float16))
        for c in range(NCORES)
    ]
    wt_shards = [np.ascontiguousarray(ws.T) for ws in w_shards]
    benc_shards = [
        np.ascontiguousarray(
            b_enc[c * HS : (c + 1) * HS].reshape(HS // 128, 128).T
        )
        for c in range(NCORES)
    ]

    # ---- launch 1: encode ----
    enc_in = [
        {"xT": xT, "w": w_shards[c], "benc": benc_shards[c]} for c in range(NCORES)
    ]
    enc_out = _run_spmd(_get("enc"), enc_in)
    f22 = [enc_out[c]["f22"] for c in range(NCORES)]
    samples = np.concatenate([enc_out[c]["samp"].ravel() for c in range(NCORES)])

    # ---- host: conservative threshold from samples ----
    margin = max(8000, n_keep // 32)
    srank = min((n_keep + margin) // SAMPLE_STRIDE, samples.size - 1)
    cut = float(np.partition(samples, samples.size - 1 - srank)[samples.size - 1 - srank])
    cut -= 5e-4
    cut = max(cut, 1e-30)

    for _attempt in range(4):
        out = _decode_and_fix(x, W, b_dec, n_keep, f22, wt_shards, cut)
        if out is not None:
            return out
        cut *= 0.25  # too few survivors; retry with a much lower threshold
    raise RuntimeError("batch top-k threshold selection failed")


def _decode_and_fix(x, W, b_dec, n_keep, f22, wt_shards, cut):
    cut_arr = np.full((128, 1), cut, dtype=np.float32)
    dec_in = [
        {"f22": f22[c], "wt": wt_shards[c], "cutv": cut_arr} for c in range(NCORES)
    ]
    dec_out = _run_spmd(_get("dec"), dec_in)
    # [HS, B] h-major shards (copy: PJRT buffers are read-only)
    ft = [np.array(dec_out[c]["ftopk"]) for c in range(NCORES)]
    recon = dec_out[0]["recon"].astype(np.float32, copy=True)
    for c in range(1, NCORES):
        recon += dec_out[c]["recon"]

    # ---- host: exact top-n_keep boundary fixup ----
    nz_h, nz_b, nz_v, nz_c = [], [], [], []
    for c in range(NCORES):
        hh, bb = np.nonzero(ft[c])
        nz_h.append(hh)
        nz_b.append(bb)
        nz_v.append(ft[c][hh, bb])
        nz_c.append(np.full(hh.shape, c, dtype=np.int32))
    v = np.concatenate(nz_v)
    m = v.size
    if m < n_keep:
        return None
    hl = np.concatenate(nz_h)
    bl = np.concatenate(nz_b)
    cl = np.concatenate(nz_c)
    hg = cl.astype(np.int64) * HS + hl  # global hidden index

    t = float(np.partition(v, m - n_keep)[m - n_keep])  # n-th largest f22 value
    WIN = 4e-4  # ~10 sigma of the fp16-input matmul deviation from fp32
    if not (cut <= t - WIN):
        return None
    definite = v > t + WIN
    n_def = int(definite.sum())
    bmask = ~definite & (v >= t - WIN)
    n_bnd = int(bmask.sum())
    if n_def > n_keep or n_def + n_bnd < n_keep:
        return None

    # recompute boundary candidates exactly; order like the reference fp32 topk
    bi = np.nonzero(bmask)[0]
    x64 = x.astype(np.float64)
    W64 = W.astype(np.float64)
    exact = np.empty(bi.size, dtype=np.float64)
    for j, i in enumerate(bi):
        exact[j] = np.dot(x64[bl[i]], W64[:, hg[i]])
    exact32 = np.maximum(exact, 0.0).astype(np.float32)
    flat_idx = bl[bi].astype(np.int64) * H + hg[bi]
    order = np.lexsort((flat_idx, -exact32.astype(np.float64)))
    keep_bnd = bi[order[: n_keep - n_def]]

    keep = np.zeros(m, dtype=bool)
    keep[definite] = True
    keep[keep_bnd] = True
    drop = np.nonzero(~keep)[0]

    # kept boundary entries get the exactly recomputed fp32 values
    exact_vals = np.zeros(m, dtype=np.float32)
    exact_vals[bi] = exact32

    # zero dropped entries / patch kept-boundary entries in the f_topk shards
    for c in range(NCORES):
        sel = drop[cl[drop] == c]
        if sel.size:
            ft[c][hl[sel], bl[sel]] = 0.0
        sel2 = keep_bnd[cl[keep_bnd] == c]
        if sel2.size:
            ft[c][hl[sel2], bl[sel2]] = exact_vals[sel2]

    # subtract dropped rank-1 contributions from recon
    if drop.size:
        contrib = v[drop][:, None].astype(np.float32) * W[:, hg[drop]].T
        np.add.at(recon, bl[drop], -contrib)

    recon += b_dec[None, :]

    f_topk = np.empty((B, H), dtype=np.float32)
    for c in range(NCORES):
        f_topk[:, c * HS : (c + 1) * HS] = ft[c].T
    return recon, f_topk
